# revision 10
# baseline (speedup 1.0000x reference)
"""Trainium2 Bass kernel for nn_Attention_47459388621522.

Computation (B=256, N=2048, D=256):
    hidden = concat([feature, broadcast(pointer_hidden_state)], -1)   # [B,N,2D]
    pre    = tanh(einsum('de,bne->bnd', W[0], hidden))                # [B,N,D]
    scores = einsum('d,bnd->bn', v[0,0], pre)                         # [B,N]
    attns  = softmax(scores, axis=1)[:, None, :]                      # [B,1,N]

Split W = [Wf | Wh] along e: pre = tanh(feature @ Wf^T + bias_b) with
bias = pointer_hidden_state @ Wh^T computed on-device in exact fp32 (tiny).

Sharding: data-parallel over batch, 32 batches per core x 8 cores.

Design (f16 + column-tiled v-dot): feature/Wf/v cast to fp16 on the host,
halving DMA vs an f32r variant (32 MB/core, ~96 us floor; PE speed is the
same since f32r already runs 1 col/cycle).  Accuracy: single-W f16 measures
rel_l2 3.7e-3 vs the fp32 reference (host-numpy prediction matched HW).

Per-core loop, one batch (2048 tokens) at a time:
    PE : pre[d,t] psum [128,1024] x2 per batch (8 MMs of 512 cols, f16)
         = 109.2 us total (FLOP-bound at 78.6 TF/s)
    ACT: th[d,t] = tanh(pre + bias[d,b]) f16, FD=1024 per instr; with the
         ~222-cycle SBUF access penalty per instr this is ~133 us and is
         the binding engine
    PE : scores via 4 column-tiled MMs per dc (tile_position=(0,32j), j=0..3)
         running CONCURRENTLY on disjoint column groups; stationary is v
         zero-padded to column `brow` so batch brow lands on psum partition
         32j+brow; the bank is pre-zeroed by a [128,512] zeros matmul
         (start=True) so every v-MM is a pure accumulate.  This cuts the
         v-dot from ~55 us of serialized PE streaming to ~15 us.
    DVE: one whole-bank [128,512] copy per 16-batch half, then 4 contiguous
         gather DMAs -> scores_half [16,2048]; softmax per half (DVE max,
         ACT exp+accum, DVE reciprocal+scale) overlaps the main loop.

Measured (slope method, R56/R112): 142.5 us vs 212.7 us for the staged f32r
baseline on the same harness.  vdot="stream" keeps the serialized v-dot as a
fallback; mode="no_tanh"/"no_vdot" are timing probes (wrong results).
"""

import numpy as np

import concourse.bacc as bacc
import concourse.mybir as mybir
import concourse.tile as tile
from concourse.bass_utils import run_bass_kernel_spmd

f32 = mybir.dt.float32
f32r = mybir.dt.float32r
f16 = mybir.dt.float16

B, N, D = 256, 2048, 256
N_CORES = 8
B_PER = B // N_CORES          # 32 batches per core
TG = 1024                     # token group (ACT free dim; 2 psum banks)
NG = N // TG                  # 2 groups per batch
P = 128
DC = D // P                   # 2 d-chunks
KC = D // P                   # 2 e-chunks
TOKS = B_PER * N              # tokens per core
HB = B_PER // 2               # batches per scores half

VDOT = "col"                  # "col" (tile_position) or "stream"

_CACHED = {}


def _build(repeat=1, ft_bufs=4, ch_tok=2048, th_bufs=3, mmps_bufs=3,
           mode="full", ft_queues="sa", vdot=None):
    # ch_tok: tokens per feature DMA chunk (multiple of N).
    # ft_queues: DMA channels for the feature load, round-robin over chunks.
    #            s=SP-HWDGE, a=ACT-HWDGE, p=Pool-SWDGE.
    vdot = vdot or VDOT
    assert ch_tok % N == 0 and TOKS % ch_tok == 0
    bat_per_ch = ch_tok // N

    nc = bacc.Bacc("TRN2", target_bir_lowering=False, debug=False, name="ptrattn")
    featT = nc.dram_tensor("featT", [D, TOKS], f16, kind="ExternalInput")
    hT = nc.dram_tensor("hT", [D, B_PER], f32, kind="ExternalInput")
    wfT = nc.dram_tensor("wfT", [D, D], f16, kind="ExternalInput")
    whT = nc.dram_tensor("whT", [D, D], f32, kind="ExternalInput")
    vv = nc.dram_tensor("vv", [D, 1], f16, kind="ExternalInput")
    out = nc.dram_tensor("attns", [B_PER, N], f32, kind="ExternalOutput")

    act = mybir.ActivationFunctionType

    with tile.TileContext(nc) as tc:
        with tc.tile_pool(name="singles", bufs=1) as singles, \
             tc.tile_pool(name="feat", bufs=ft_bufs) as feat_pool, \
             tc.tile_pool(name="th", bufs=th_bufs) as th_pool, \
             tc.tile_pool(name="stage", bufs=3) as stage_pool, \
             tc.tile_pool(name="soft", bufs=1) as soft_pool, \
             tc.tile_pool(name="mmps", bufs=mmps_bufs, space="PSUM") as mmps, \
             tc.tile_pool(name="scps", bufs=1, space="PSUM") as scps:

            # ---- constants (bias inputs first so bias is ready earliest) ----
            wh_full = singles.tile([P, KC, D], f32)
            nc.sync.dma_start(wh_full, whT.rearrange("(ko p) d -> p ko d", p=P))
            hT_sb = singles.tile([P, KC, B_PER], f32)
            nc.sync.dma_start(hT_sb, hT.rearrange("(ko p) b -> p ko b", p=P))
            wf_sb = singles.tile([P, KC, D], f16)
            nc.sync.dma_start(wf_sb, wfT.rearrange("(ko p) d -> p ko d", p=P))
            # zero-padded v: vpad[:, dc, 0:32] = 0, vpad[:, dc, 32] = v chunk
            vpad = singles.tile([P, DC, 33], f16)
            nc.vector.memset(vpad, 0.0)
            nc.sync.dma_start(
                vpad[:, :, 32:33], vv.rearrange("(ko p) one -> p ko one", p=P))
            zpad = singles.tile([P, 512], f16)
            nc.vector.memset(zpad, 0.0)

            # ---- bias[b, d] = Wh @ h_b  (exact fp32, tiny) ----
            bias_sb = singles.tile([P, DC, B_PER], f32)
            for dc in range(DC):
                bias_ps = mmps.tile([P, TG], f32, tag="pre", bufs=None)
                for ko in range(KC):
                    nc.tensor.matmul(
                        bias_ps[:, :B_PER],
                        wh_full[:, ko, dc * P:(dc + 1) * P],
                        hT_sb[:, ko, :],
                        start=(ko == 0), stop=(ko == KC - 1),
                    )
                nc.vector.tensor_copy(bias_sb[:, dc, :], bias_ps[:, :B_PER])

            # scores accumulators, two halves so softmax(half0) overlaps the
            # main loop (DVE ops need base-partition 0, so separate tiles)
            scores_half = [soft_pool.tile([HB, N], f32, name=f"scores{h}", tag=f"scores{h}")
                           for h in range(2)]
            if vdot == "col":
                # psum score banks for "col" vdot: partition 32j+b, cols =
                # tokens of quarter j; one bank per 16-batch half
                sc_banks = [scps.tile([P, 512], f32, name=f"scb{h}", tag=f"scb{h}")
                            for h in range(2)]
                sc_sb = [stage_pool.tile([P, 512], f32, name=f"scsb{h}",
                                         tag=f"scsb{h}", bufs=1)
                         for h in range(2)]

            def softmax_half(h):
                scores = scores_half[h]
                negmax = soft_pool.tile([HB, 1], f32, tag=f"negmax{h}")
                nc.vector.tensor_reduce(
                    negmax, scores, axis=mybir.AxisListType.X,
                    op=mybir.AluOpType.max, negate=True)
                # exp(score - max) fused via per-partition bias; the ACT exp
                # LUT underflows cleanly to 0 for very negative inputs
                probs = soft_pool.tile([HB, N], f32, tag=f"probs{h}")
                sumexp = soft_pool.tile([HB, 1], f32, tag=f"sumexp{h}")
                nc.scalar.activation(
                    probs, scores, act.Exp, bias=negmax, scale=1.0,
                    accum_out=sumexp)
                rcp = soft_pool.tile([HB, 1], f32, tag=f"rcp{h}")
                nc.vector.reciprocal(rcp, sumexp)
                nc.vector.tensor_scalar_mul(probs, probs, rcp)
                nc.gpsimd.dma_start(out.ap()[h * HB:(h + 1) * HB, :], probs)

            # ---- main loop over feature chunks ----
            qmap = {"s": nc.sync, "a": nc.scalar, "p": nc.gpsimd}
            featT_r = featT.rearrange("(ko p) t -> p ko t", p=P)
            for rep in range(repeat):
                for ch in range(TOKS // ch_tok):
                    ft = feat_pool.tile([P, KC, ch_tok], f16, tag="ft")
                    eng = qmap[ft_queues[ch % len(ft_queues)]]
                    ft_src = featT_r[:, :, ch * ch_tok:(ch + 1) * ch_tok]
                    if ch == 0 and rep == 0:
                        # split the first load so the pipeline starts on the
                        # first quarter instead of waiting for the full chunk
                        q = ch_tok // 4
                        for s in range(4):
                            eng.dma_start(ft[:, :, s * q:(s + 1) * q],
                                          ft_src[:, :, s * q:(s + 1) * q])
                    else:
                        eng.dma_start(ft, ft_src)

                    for bl in range(bat_per_ch):
                        b = ch * bat_per_ch + bl
                        h, brow = divmod(b, HB)
                        if brow == 0 and vdot == "col":
                            # zero the whole score bank (start=True writes 0
                            # everywhere and sets has_written uniformly); all
                            # batch v-MMs below are then pure accumulates.
                            nc.tensor.matmul(
                                sc_banks[h], zpad[:, 0:128], zpad,
                                start=True, stop=(mode == "no_vdot"),
                                skip_group_check=True)
                        if mode == "dma_only":
                            stage = stage_pool.tile([1, N], f32, tag="stage")
                            nc.vector.tensor_copy(stage[:, 0:8], ft[0:1, 0, 0:8])
                            nc.gpsimd.dma_start(
                                scores_half[h][brow:brow + 1, 0:2], stage[:, 0:2])
                            if brow == HB - 1:
                                softmax_half(h)
                            continue
                        th = th_pool.tile([P, DC, N], f16, tag="th")
                        for g in range(NG):
                            ts = slice(bl * N + g * TG, bl * N + (g + 1) * TG)
                            for dc in range(DC):
                                pre = mmps.tile([P, TG], f32, tag="pre")
                                for ko in range(KC):
                                    for half in range(TG // 512):
                                        cs = slice(half * 512, (half + 1) * 512)
                                        tsc = slice(ts.start + half * 512,
                                                    ts.start + (half + 1) * 512)
                                        nc.tensor.matmul(
                                            pre[:, cs],
                                            wf_sb[:, ko, dc * P:(dc + 1) * P],
                                            ft[:, ko, tsc],
                                            start=(ko == 0), stop=(ko == KC - 1),
                                        )
                                if mode != "no_tanh":
                                    nc.scalar.activation(
                                        th[:, dc, g * TG:(g + 1) * TG], pre,
                                        act.Tanh,
                                        bias=bias_sb[:, dc, b:b + 1], scale=1.0)
                        if vdot == "col":
                            # scores for batch b: 4 column-tiled MMs per dc,
                            # concurrent across column groups j; batch lands
                            # on psum partition 32j + brow via the zero-pad
                            # trick (stationary [128, brow+1], v in last col)
                            for dc in range(DC):
                                for j in range(4):
                                    if mode == "no_vdot":
                                        continue
                                    rhs = (ft[:, 0, bl * N + 512 * j:
                                              bl * N + 512 * (j + 1)]
                                           if mode == "no_tanh" else
                                           th[:, dc, 512 * j:512 * (j + 1)])
                                    last = (brow == HB - 1 and dc == DC - 1
                                            and j == 3)
                                    nc.tensor.matmul(
                                        sc_banks[h][32 * j:32 * j + brow + 1, :],
                                        vpad[:, dc, 32 - brow:33],
                                        rhs,
                                        start=False, stop=last,
                                        skip_group_check=True,
                                        tile_position=(0, 32 * j),
                                    )
                            if brow == HB - 1:
                                nc.vector.tensor_copy(sc_sb[h], sc_banks[h])
                                # gather [16, 2048]: batch row brow comes from
                                # partitions {32j+brow}, 512 cols each
                                for j in range(4):
                                    nc.gpsimd.dma_start(
                                        scores_half[h][:, 512 * j:512 * (j + 1)],
                                        sc_sb[h][32 * j:32 * j + HB, :])
                                softmax_half(h)
                        else:
                            stage = stage_pool.tile([1, N], f32, tag="stage")
                            for g2 in range(N // 512):
                                sc = scps.tile([1, 512], f32, tag="sc", bufs=2)
                                for dc in range(DC):
                                    nc.tensor.matmul(
                                        sc, vpad[:, dc, 32:33],
                                        th[:, dc, 512 * g2:512 * (g2 + 1)],
                                        start=(dc == 0), stop=(dc == DC - 1),
                                    )
                                nc.vector.tensor_copy(
                                    stage[:, 512 * g2:512 * (g2 + 1)], sc)
                            nc.gpsimd.dma_start(
                                scores_half[h][brow:brow + 1, :], stage)
                            if brow == HB - 1:
                                softmax_half(h)

    nc.compile()
    return nc


def _host_prep(feature, pointer_hidden_state, v, W):
    Wf = W[0][:, :D]
    whT = np.ascontiguousarray(W[0][:, D:].T.astype(np.float32))       # [e, d]
    wfT = np.ascontiguousarray(Wf.T.astype(np.float16))                # [e, d]
    vv = np.ascontiguousarray(v[0, 0][:, None].astype(np.float16))    # [D, 1]
    per_core = []
    for c in range(N_CORES):
        sl = slice(c * B_PER, (c + 1) * B_PER)
        # [D, B_PER*N] global token stream: featT[e, b*N+n] = feature[b, n, e]
        featT = np.ascontiguousarray(
            feature[sl].astype(np.float16).transpose(2, 0, 1).reshape(D, TOKS))
        hT = np.ascontiguousarray(pointer_hidden_state[sl].T.astype(np.float32))
        per_core.append({"featT": featT, "hT": hT, "wfT": wfT, "whT": whT, "vv": vv})
    return per_core


def kernel(feature, pointer_hidden_state, v, W):
    feature = np.asarray(feature)
    pointer_hidden_state = np.asarray(pointer_hidden_state)
    v = np.asarray(v)
    W = np.asarray(W)

    if "nc" not in _CACHED:
        _CACHED["nc"] = _build()
    nc = _CACHED["nc"]

    in_maps = _host_prep(feature, pointer_hidden_state, v, W)
    res = run_bass_kernel_spmd(nc, in_maps, core_ids=list(range(N_CORES)))
    _CACHED["last_res"] = res
    outs = [res.results[c]["attns"] for c in range(N_CORES)]
    return np.concatenate(outs, axis=0)[:, None, :].astype(np.float32)


# revision 15
# speedup vs baseline: 1.1100x; 1.1100x over previous
"""Trainium2 Bass kernel for nn_Attention_47459388621522.

Computation (B=256, N=2048, D=256):
    hidden = concat([feature, broadcast(pointer_hidden_state)], -1)   # [B,N,2D]
    pre    = tanh(einsum('de,bne->bnd', W[0], hidden))                # [B,N,D]
    scores = einsum('d,bnd->bn', v[0,0], pre)                         # [B,N]
    attns  = softmax(scores, axis=1)[:, None, :]                      # [B,1,N]

Split W = [Wf | Wh] along e: pre = tanh(feature @ Wf^T + bias_b) with
bias = pointer_hidden_state @ Wh^T computed on-device in exact fp32 (tiny).

Sharding: data-parallel over batch, 32 batches per core x 8 cores.

Design (f16 + column-tiled v-dot): feature/Wf/v cast to fp16 on the host,
halving DMA vs an f32r variant (32 MB/core, ~96 us floor; PE speed is the
same since f32r already runs 1 col/cycle).  Accuracy: single-W f16 measures
rel_l2 3.7e-3 vs the fp32 reference (host-numpy prediction matched HW).

Per-core loop, one batch (2048 tokens) at a time:
    PE : pre[d,t] psum [128,1024] x2 per batch (8 MMs of 512 cols, f16)
         = 109.2 us total (FLOP-bound at 78.6 TF/s)
    ACT: th[d,t] = tanh(pre + bias[d,b]) f16, FD=1024 per instr; with the
         ~222-cycle SBUF access penalty per instr this is ~133 us and is
         the binding engine
    PE : scores via 4 column-tiled MMs per dc (tile_position=(0,32j), j=0..3)
         running CONCURRENTLY on disjoint column groups; stationary is v
         zero-padded to column `brow` so batch brow lands on psum partition
         32j+brow; the bank is pre-zeroed by a [128,512] zeros matmul
         (start=True) so every v-MM is a pure accumulate.  This cuts the
         v-dot from ~55 us of serialized PE streaming to ~15 us.
    DVE: one whole-bank [128,512] copy per 16-batch half, then 4 contiguous
         gather DMAs -> scores_half [16,2048]; softmax per half (DVE max,
         ACT exp+accum, DVE reciprocal+scale) overlaps the main loop.

Measured (slope method, R56/R112): 142.5 us vs 212.7 us for the staged f32r
baseline on the same harness.  vdot="stream" keeps the serialized v-dot as a
fallback; mode="no_tanh"/"no_vdot" are timing probes (wrong results).
"""

import numpy as np

import concourse.bacc as bacc
import concourse.mybir as mybir
import concourse.tile as tile
from concourse.bass_utils import run_bass_kernel_spmd

f32 = mybir.dt.float32
f32r = mybir.dt.float32r
f16 = mybir.dt.float16

B, N, D = 256, 2048, 256
N_CORES = 8
B_PER = B // N_CORES          # 32 batches per core
TG = 1024                     # token group (ACT free dim; 2 psum banks)
NG = N // TG                  # 2 groups per batch
P = 128
DC = D // P                   # 2 d-chunks
KC = D // P                   # 2 e-chunks
TOKS = B_PER * N              # tokens per core
HB = B_PER // 2               # batches per scores half

VDOT = "col"                  # "col" (tile_position) or "stream"

_CACHED = {}


def _build(repeat=1, ft_bufs=3, ch_tok=2048, th_bufs=2, mmps_bufs=3,
           mode="full", ft_queues="sa", vdot=None):
    # ch_tok: tokens per feature DMA chunk (multiple of N).
    # ft_queues: DMA channels for the feature load, round-robin over chunks.
    #            s=SP-HWDGE, a=ACT-HWDGE, p=Pool-SWDGE.
    vdot = vdot or VDOT
    assert ch_tok % N == 0 and TOKS % ch_tok == 0
    bat_per_ch = ch_tok // N

    nc = bacc.Bacc("TRN2", target_bir_lowering=False, debug=False, name="ptrattn")
    featT = nc.dram_tensor("featT", [D, TOKS], f16, kind="ExternalInput")
    hT = nc.dram_tensor("hT", [D, B_PER], f32, kind="ExternalInput")
    wfT = nc.dram_tensor("wfT", [D, D], f16, kind="ExternalInput")
    whT = nc.dram_tensor("whT", [D, D], f32, kind="ExternalInput")
    vv = nc.dram_tensor("vv", [D, 1], f16, kind="ExternalInput")
    out = nc.dram_tensor("attns", [B_PER, N], f32, kind="ExternalOutput")

    act = mybir.ActivationFunctionType

    with tile.TileContext(nc) as tc:
        with tc.tile_pool(name="singles", bufs=1) as singles, \
             tc.tile_pool(name="feat", bufs=ft_bufs) as feat_pool, \
             tc.tile_pool(name="th", bufs=th_bufs) as th_pool, \
             tc.tile_pool(name="stage", bufs=3) as stage_pool, \
             tc.tile_pool(name="soft", bufs=1) as soft_pool, \
             tc.tile_pool(name="mmps", bufs=mmps_bufs, space="PSUM") as mmps, \
             tc.tile_pool(name="scps", bufs=1, space="PSUM") as scps:

            # ---- constants (bias inputs first so bias is ready earliest) ----
            wh_full = singles.tile([P, KC, D], f32)
            nc.sync.dma_start(wh_full, whT.rearrange("(ko p) d -> p ko d", p=P))
            hT_sb = singles.tile([P, KC, B_PER], f32)
            nc.sync.dma_start(hT_sb, hT.rearrange("(ko p) b -> p ko b", p=P))
            wf_sb = singles.tile([P, KC, D], f16)
            nc.sync.dma_start(wf_sb, wfT.rearrange("(ko p) d -> p ko d", p=P))
            # zero-padded v: vpad[:, dc, 0:32] = 0, vpad[:, dc, 32] = v chunk
            vpad = singles.tile([P, DC, 33], f16)
            nc.vector.memset(vpad, 0.0)
            nc.sync.dma_start(
                vpad[:, :, 32:33], vv.rearrange("(ko p) one -> p ko one", p=P))
            zpad = singles.tile([P, 512], f16)
            nc.vector.memset(zpad, 0.0)

            # ---- bias[b, d] = Wh @ h_b  (exact fp32, tiny) ----
            bias_sb = singles.tile([P, DC, B_PER], f32)
            for dc in range(DC):
                bias_ps = mmps.tile([P, TG], f32, tag="pre", bufs=None)
                for ko in range(KC):
                    nc.tensor.matmul(
                        bias_ps[:, :B_PER],
                        wh_full[:, ko, dc * P:(dc + 1) * P],
                        hT_sb[:, ko, :],
                        start=(ko == 0), stop=(ko == KC - 1),
                    )
                nc.vector.tensor_copy(bias_sb[:, dc, :], bias_ps[:, :B_PER])

            # scores accumulators, two halves so softmax(half0) overlaps the
            # main loop (DVE ops need base-partition 0, so separate tiles)
            scores_half = [soft_pool.tile([HB, N], f32, name=f"scores{h}", tag=f"scores{h}")
                           for h in range(2)]
            if vdot == "col":
                # psum score banks for "col" vdot: partition 32j+b, cols =
                # tokens of quarter j; one bank per 16-batch half
                sc_banks = [scps.tile([P, 512], f32, name=f"scb{h}", tag=f"scb{h}")
                            for h in range(2)]
                sc_sb = [stage_pool.tile([P, 512], f32, name=f"scsb{h}",
                                         tag=f"scsb{h}", bufs=1)
                         for h in range(2)]

            def softmax_half(h):
                scores = scores_half[h]
                negmax = soft_pool.tile([HB, 1], f32, tag=f"negmax{h}")
                nc.vector.tensor_reduce(
                    negmax, scores, axis=mybir.AxisListType.X,
                    op=mybir.AluOpType.max, negate=True)
                # exp(score - max) fused via per-partition bias; the ACT exp
                # LUT underflows cleanly to 0 for very negative inputs
                probs = soft_pool.tile([HB, N], f32, tag=f"probs{h}")
                sumexp = soft_pool.tile([HB, 1], f32, tag=f"sumexp{h}")
                nc.scalar.activation(
                    probs, scores, act.Exp, bias=negmax, scale=1.0,
                    accum_out=sumexp)
                rcp = soft_pool.tile([HB, 1], f32, tag=f"rcp{h}")
                nc.vector.reciprocal(rcp, sumexp)
                nc.vector.tensor_scalar_mul(probs, probs, rcp)
                nc.gpsimd.dma_start(out.ap()[h * HB:(h + 1) * HB, :], probs)

            def flush_v(pend):
                # deferred v-dot for batch pb: emitted AFTER batch pb+1's
                # pre-matmuls so the in-order PE queue never stalls at a
                # v-MM waiting on ACT's th (head-of-line blocking)
                pb, pth, pft, pbl = pend
                ph, pbrow = divmod(pb, HB)
                if pbrow == 0:
                    # zero the whole score bank (start=True writes 0
                    # everywhere and sets has_written uniformly); all
                    # batch v-MMs below are then pure accumulates.
                    nc.tensor.matmul(
                        sc_banks[ph], zpad[:, 0:128], zpad,
                        start=True, stop=False, skip_group_check=True)
                for dc in range(DC):
                    for j in range(4):
                        rhs = (pft[:, 0, pbl * N + 512 * j:
                                   pbl * N + 512 * (j + 1)]
                               if mode == "no_tanh" else
                               pth[:, dc, 512 * j:512 * (j + 1)])
                        last = (pbrow == HB - 1 and dc == DC - 1 and j == 3)
                        nc.tensor.matmul(
                            sc_banks[ph][32 * j:32 * j + pbrow + 1, :],
                            vpad[:, dc, 32 - pbrow:33],
                            rhs,
                            start=False, stop=last,
                            skip_group_check=True,
                            tile_position=(0, 32 * j),
                        )
                if pbrow == HB - 1:
                    nc.vector.tensor_copy(sc_sb[ph], sc_banks[ph])
                    # gather [16, 2048]: batch row comes from partitions
                    # {32j+row}, 512 contiguous cols each
                    for j in range(4):
                        nc.gpsimd.dma_start(
                            scores_half[ph][:, 512 * j:512 * (j + 1)],
                            sc_sb[ph][32 * j:32 * j + HB, :])
                    softmax_half(ph)

            # ---- main loop over feature chunks ----
            qmap = {"s": nc.sync, "a": nc.scalar, "p": nc.gpsimd}
            featT_r = featT.rearrange("(ko p) t -> p ko t", p=P)
            for rep in range(repeat):
                pending = None
                for ch in range(TOKS // ch_tok):
                    ft = feat_pool.tile([P, KC, ch_tok], f16, tag="ft")
                    eng = qmap[ft_queues[ch % len(ft_queues)]]
                    ft_src = featT_r[:, :, ch * ch_tok:(ch + 1) * ch_tok]
                    if ch == 0 and rep == 0:
                        # split the first load so the pipeline starts on the
                        # first quarter instead of waiting for the full chunk
                        q = ch_tok // 4
                        for s in range(4):
                            eng.dma_start(ft[:, :, s * q:(s + 1) * q],
                                          ft_src[:, :, s * q:(s + 1) * q])
                    else:
                        eng.dma_start(ft, ft_src)

                    for bl in range(bat_per_ch):
                        b = ch * bat_per_ch + bl
                        h, brow = divmod(b, HB)
                        if brow == 0 and vdot == "col" and mode == "no_vdot":
                            nc.tensor.matmul(
                                sc_banks[h], zpad[:, 0:128], zpad,
                                start=True, stop=True,
                                skip_group_check=True)
                        if mode == "dma_only":
                            stage = stage_pool.tile([1, N], f32, tag="stage")
                            nc.vector.tensor_copy(stage[:, 0:8], ft[0:1, 0, 0:8])
                            nc.gpsimd.dma_start(
                                scores_half[h][brow:brow + 1, 0:2], stage[:, 0:2])
                            if brow == HB - 1:
                                softmax_half(h)
                            continue
                        th = th_pool.tile([P, DC, N], f16, tag="th")
                        for g in range(NG):
                            ts = slice(bl * N + g * TG, bl * N + (g + 1) * TG)
                            for dc in range(DC):
                                pre = mmps.tile([P, TG], f32, tag="pre")
                                for ko in range(KC):
                                    for half in range(TG // 512):
                                        cs = slice(half * 512, (half + 1) * 512)
                                        tsc = slice(ts.start + half * 512,
                                                    ts.start + (half + 1) * 512)
                                        nc.tensor.matmul(
                                            pre[:, cs],
                                            wf_sb[:, ko, dc * P:(dc + 1) * P],
                                            ft[:, ko, tsc],
                                            start=(ko == 0), stop=(ko == KC - 1),
                                        )
                                if mode != "no_tanh":
                                    nc.scalar.activation(
                                        th[:, dc, g * TG:(g + 1) * TG], pre,
                                        act.Tanh,
                                        bias=bias_sb[:, dc, b:b + 1], scale=1.0)
                        if vdot == "col":
                            if mode == "no_vdot":
                                if brow == HB - 1:
                                    nc.vector.tensor_copy(sc_sb[h], sc_banks[h])
                                    for j in range(4):
                                        nc.gpsimd.dma_start(
                                            scores_half[h][:, 512 * j:512 * (j + 1)],
                                            sc_sb[h][32 * j:32 * j + HB, :])
                                    softmax_half(h)
                            else:
                                # defer batch b's v-dot until after batch
                                # b+1's pre-matmuls (see flush_v)
                                if pending is not None:
                                    flush_v(pending)
                                pending = (b, th, ft, bl)
                        else:
                            stage = stage_pool.tile([1, N], f32, tag="stage")
                            for g2 in range(N // 512):
                                sc = scps.tile([1, 512], f32, tag="sc", bufs=2)
                                for dc in range(DC):
                                    nc.tensor.matmul(
                                        sc, vpad[:, dc, 32:33],
                                        th[:, dc, 512 * g2:512 * (g2 + 1)],
                                        start=(dc == 0), stop=(dc == DC - 1),
                                    )
                                nc.vector.tensor_copy(
                                    stage[:, 512 * g2:512 * (g2 + 1)], sc)
                            nc.gpsimd.dma_start(
                                scores_half[h][brow:brow + 1, :], stage)
                            if brow == HB - 1:
                                softmax_half(h)
                if pending is not None:
                    flush_v(pending)
                    pending = None

    nc.compile()
    return nc


def _host_prep(feature, pointer_hidden_state, v, W):
    Wf = W[0][:, :D]
    whT = np.ascontiguousarray(W[0][:, D:].T.astype(np.float32))       # [e, d]
    wfT = np.ascontiguousarray(Wf.T.astype(np.float16))                # [e, d]
    vv = np.ascontiguousarray(v[0, 0][:, None].astype(np.float16))    # [D, 1]
    per_core = []
    for c in range(N_CORES):
        sl = slice(c * B_PER, (c + 1) * B_PER)
        # [D, B_PER*N] global token stream: featT[e, b*N+n] = feature[b, n, e]
        featT = np.ascontiguousarray(
            feature[sl].astype(np.float16).transpose(2, 0, 1).reshape(D, TOKS))
        hT = np.ascontiguousarray(pointer_hidden_state[sl].T.astype(np.float32))
        per_core.append({"featT": featT, "hT": hT, "wfT": wfT, "whT": whT, "vv": vv})
    return per_core


def kernel(feature, pointer_hidden_state, v, W):
    feature = np.asarray(feature)
    pointer_hidden_state = np.asarray(pointer_hidden_state)
    v = np.asarray(v)
    W = np.asarray(W)

    if "nc" not in _CACHED:
        _CACHED["nc"] = _build()
    nc = _CACHED["nc"]

    in_maps = _host_prep(feature, pointer_hidden_state, v, W)
    res = run_bass_kernel_spmd(nc, in_maps, core_ids=list(range(N_CORES)))
    _CACHED["last_res"] = res
    outs = [res.results[c]["attns"] for c in range(N_CORES)]
    return np.concatenate(outs, axis=0)[:, None, :].astype(np.float32)
